# revision 1
# baseline (speedup 1.0000x reference)
"""Trainium2 Bass kernel for GNN aggregate-update (scatter-mean + concat + MLP).

Strategy (8 NeuronCores, SPMD, no collectives):
  - Host (sharding/routing only): sort edge ids by target node and route each
    edge's feature row to the core that owns its target (cores own contiguous
    1/8 node ranges). Each core's edges land in one contiguous bf16 buffer,
    grouped by 64-node block; all 8 blocks of a 512-node MLP group share one
    padded per-block capacity so the whole group loads with a single DMA.
  - Device, per core: per group, ONE strided DMA (alternating between the two
    HWDGE rings) loads 8 blocks of edges so each SBUF partition holds a
    contiguous run of edge rows per block (multi-KB descriptors -> line-rate
    HBM reads). Scatter-mean becomes dense one-hot matmuls: per block, ONE
    DVE tensor_tensor(is_equal) builds the block's one-hot
    [128e, Jg*64n] bf16 (tiled-iota constant vs per-slot local-target scalars
    via a stride-0 broadcast AP); the PE accumulates
    aggT[f, n] += attr_chunk.T @ onehot_chunk into a per-group PSUM bank.
    64-node blocks halve the DVE one-hot work vs 128-node blocks.
    recip = 1/max(degree,1) is replicated across partitions by a K=1 PE
    matmul (ones.T @ recip_row), copied to SBUF by ACT, and applied by one
    DVE multiply per group while evicting the aggregate PSUM->SBUF.
  - MLP in transposed layout, features on partitions: y1T = relu(W1T.T @
    [xT; aggT] + b1), y2T = W2T.T @ y1T + b2, biases applied by the ACT
    engine at PSUM eviction. MLP operands in bf16 (PSUM accumulation stays
    fp32); final output is fp32. Output stays transposed [128, nodes]; the
    host transposes back while unsharding.
"""

import numpy as np
import ml_dtypes

N_NODES = 100_000
N_EDGES = 1_600_000
F = 128
HIDDEN = 256
OUT_F = 128
N_CORES = 8
P = 128
NODES_PER_CORE = N_NODES // N_CORES          # 12500
NODE_B = 64                                  # nodes per aggregation block
BLOCKS = -(-NODES_PER_CORE // NODE_B)        # 196
GROUP_NODES = 512                            # MLP group width
GROUP_BLOCKS = GROUP_NODES // NODE_B         # 8 blocks per group
N_GROUPS = -(-BLOCKS // GROUP_BLOCKS)        # 25 (last group partial)
NLOC = BLOCKS * NODE_B                       # 12544
MLP_BF16 = True

BF16 = ml_dtypes.bfloat16

_COMPILED = {}
LAST_EXEC_NS = None
LAST_RESULTS = None


def _preprocess(x, edge_index, edge_attr, W1, b1, W2, b2):
    """Host routing: sort edge ids by target node, build per-core contiguous
    group-padded edge buffers + per-slot local-target tables."""
    col = np.asarray(edge_index[1]).astype(np.int64)
    order = np.argsort(col, kind="stable")
    sorted_col = col[order]

    counts = np.bincount(col, minlength=N_NODES)
    recip_full = (1.0 / np.maximum(counts, 1)).astype(np.float32)

    lows = np.empty(N_CORES * BLOCKS, np.int64)
    highs = np.empty(N_CORES * BLOCKS, np.int64)
    for c in range(N_CORES):
        base = c * NODES_PER_CORE
        for b in range(BLOCKS):
            i = c * BLOCKS + b
            lows[i] = base + b * NODE_B
            highs[i] = min(base + (b + 1) * NODE_B, base + NODES_PER_CORE)
    starts = np.searchsorted(sorted_col, lows, side="left")
    ends = np.searchsorted(sorted_col, highs, side="left")
    n_cb = (ends - starts).reshape(N_CORES, BLOCKS)

    # per-GROUP uniform 128-edge chunk count (shared across cores + blocks of
    # the group, so a group's 8 blocks form one rectangular DMA)
    n_max_b = n_cb.max(axis=0)
    Jg = np.zeros(N_GROUPS, np.int64)
    for g in range(N_GROUPS):
        b0, b1_ = g * GROUP_BLOCKS, min((g + 1) * GROUP_BLOCKS, BLOCKS)
        Jg[g] = max(1, int(-(-int(n_max_b[b0:b1_].max()) // P)))
    gnb = [min(GROUP_BLOCKS, BLOCKS - g * GROUP_BLOCKS) for g in range(N_GROUPS)]
    cap_g = Jg * P                                  # rows per block in group g
    rows_g = cap_g * gnb                            # rows per group
    offg = np.zeros(N_GROUPS + 1, np.int64)
    offg[1:] = np.cumsum(rows_g)
    E_pad = int(offg[-1])

    cols_g = Jg * gnb                               # lt cols per group
    cog = np.zeros(N_GROUPS + 1, np.int64)
    cog[1:] = np.cumsum(cols_g)
    TOTC = int(cog[-1])

    ea16 = np.asarray(edge_attr, np.float32).astype(BF16)

    attr = np.zeros((N_CORES, E_pad, F), BF16)
    lt_all = np.full((N_CORES, P, TOTC), 3000.0, BF16)
    si = starts.reshape(N_CORES, BLOCKS)
    for c in range(N_CORES):
        for b in range(BLOCKS):
            g, bl = b // GROUP_BLOCKS, b % GROUP_BLOCKS
            n = int(n_cb[c, b])
            jb = int(Jg[g])
            cap = jb * P
            o = int(offg[g]) + bl * cap
            if n:
                s = int(si[c, b])
                attr[c, o:o + n] = ea16[order[s:s + n]]
                tgt = sorted_col[s:s + n]
                ltb = np.full(cap, 3000.0, np.float32)
                ltb[:n] = (tgt - lows[c * BLOCKS + b]).astype(np.float32)
                # slot r = p*jb + j  ->  [128, jb]
                co = int(cog[g]) + bl * jb
                lt_all[c, :, co:co + jb] = ltb.reshape(P, jb).astype(BF16)

    # per-core recip over padded local nodes
    recip_loc = np.ones((N_CORES, NLOC), np.float32)
    for c in range(N_CORES):
        recip_loc[c, :NODES_PER_CORE] = \
            recip_full[c * NODES_PER_CORE:(c + 1) * NODES_PER_CORE]

    mdt = BF16 if MLP_BF16 else np.float32
    xT = np.zeros((N_CORES, F, NLOC), mdt)
    xt_full = np.ascontiguousarray(np.asarray(x, np.float32).T)
    for c in range(N_CORES):
        xT[c, :, :NODES_PER_CORE] = \
            xt_full[:, c * NODES_PER_CORE:(c + 1) * NODES_PER_CORE].astype(mdt)

    w1t = np.ascontiguousarray(np.asarray(W1, np.float32).T).astype(mdt)
    w2t = np.ascontiguousarray(np.asarray(W2, np.float32).T).astype(mdt)
    Jmax = int(Jg.max())
    iota_big = np.broadcast_to(np.arange(NODE_B, dtype=np.float32),
                               (Jmax, NODE_B)).reshape(1, Jmax * NODE_B)
    iota_big = np.broadcast_to(iota_big, (P, Jmax * NODE_B)).astype(BF16)

    in_maps = []
    for c in range(N_CORES):
        in_maps.append({
            "ea": np.ascontiguousarray(attr[c]),
            "lt": np.ascontiguousarray(lt_all[c]),
            "recip": np.ascontiguousarray(recip_loc[c]),
            "xT": np.ascontiguousarray(xT[c]),
            "w1t": w1t,
            "w2t": w2t,
            "b1": np.asarray(b1, np.float32),
            "b2": np.asarray(b2, np.float32),
            "iotab": np.ascontiguousarray(iota_big),
        })
    params = tuple(int(v) for v in Jg)
    return in_maps, params


def _build(params):
    """Build + compile the per-core Bass program (same NEFF for all cores)."""
    import concourse.bass as bass
    import concourse.bacc as bacc
    import concourse.tile as tile
    import concourse.mybir as mybir

    Jg = list(params)
    f32 = mybir.dt.float32
    bf16 = mybir.dt.bfloat16
    mdt = bf16 if MLP_BF16 else f32
    gnb = [min(GROUP_BLOCKS, BLOCKS - g * GROUP_BLOCKS) for g in range(N_GROUPS)]
    cap_g = [P * j for j in Jg]
    rows_g = [cap_g[g] * gnb[g] for g in range(N_GROUPS)]
    offg = np.concatenate([[0], np.cumsum(rows_g)]).astype(int)
    E_pad = int(offg[-1])
    cols_g = [Jg[g] * gnb[g] for g in range(N_GROUPS)]
    cog = np.concatenate([[0], np.cumsum(cols_g)]).astype(int)
    TOTC = int(cog[-1])
    Jmax = max(Jg)

    nc = bacc.Bacc("TRN2", target_bir_lowering=False, debug=False,
                   num_devices=N_CORES)
    ea_d = nc.dram_tensor("ea", [E_pad, F], bf16, kind="ExternalInput").ap()
    lt_d = nc.dram_tensor("lt", [P, TOTC], bf16, kind="ExternalInput").ap()
    rc_d = nc.dram_tensor("recip", [NLOC], f32, kind="ExternalInput").ap()
    xt_d = nc.dram_tensor("xT", [F, NLOC], mdt, kind="ExternalInput").ap()
    w1t_d = nc.dram_tensor("w1t", [HIDDEN, HIDDEN], mdt, kind="ExternalInput").ap()
    w2t_d = nc.dram_tensor("w2t", [HIDDEN, OUT_F], mdt, kind="ExternalInput").ap()
    b1_d = nc.dram_tensor("b1", [HIDDEN], f32, kind="ExternalInput").ap()
    b2_d = nc.dram_tensor("b2", [OUT_F], f32, kind="ExternalInput").ap()
    io_d = nc.dram_tensor("iotab", [P, Jmax * NODE_B], bf16, kind="ExternalInput").ap()
    out_d = nc.dram_tensor("out", [OUT_F, NLOC], f32, kind="ExternalOutput").ap()

    with tile.TileContext(nc) as tc:
        with (
            tc.tile_pool(name="const", bufs=1) as cp,
            tc.tile_pool(name="tb", bufs=3) as tbp,
            tc.tile_pool(name="ga", bufs=3) as gap,
            tc.tile_pool(name="oh", bufs=6) as ohp,
            tc.tile_pool(name="mlp", bufs=2) as mp,
            tc.tile_pool(name="agg_ps", bufs=2, space="PSUM") as aps,
            tc.tile_pool(name="y1_ps", bufs=2, space="PSUM") as y1ps,
            tc.tile_pool(name="y2_ps", bufs=1, space="PSUM") as y2ps,
        ):
            # ---- constants ----
            iota_t = cp.tile([P, Jmax * NODE_B], bf16)
            nc.scalar.dma_start(out=iota_t[:], in_=io_d[:])
            w1t_t = []
            for fc in range(2):
                w1c = cp.tile([P, HIDDEN], mdt, name=f"w1c{fc}")
                nc.scalar.dma_start(out=w1c[:], in_=w1t_d[fc * P:(fc + 1) * P, :])
                w1t_t.append(w1c)
            w2t_t = []
            for oc in range(2):
                w2c = cp.tile([P, OUT_F], mdt, name=f"w2c{oc}")
                nc.scalar.dma_start(out=w2c[:], in_=w2t_d[oc * P:(oc + 1) * P, :])
                w2t_t.append(w2c)
            b1_t = []
            for oh in range(2):
                b1c = cp.tile([P, 1], f32, name=f"b1c{oh}")
                nc.scalar.dma_start(out=b1c[:], in_=b1_d[oh * P:(oh + 1) * P, None])
                b1_t.append(b1c)
            b2_t = cp.tile([P, 1], f32)
            nc.scalar.dma_start(out=b2_t[:], in_=b2_d[:, None])
            ones_t = cp.tile([1, P], f32)
            nc.vector.memset(ones_t[:], 1.0)
            rcrow_t = cp.tile([1, NLOC], f32)
            nc.scalar.dma_start(out=rcrow_t[:], in_=rc_d[None, :])

            for g in range(N_GROUPS):
                gb0 = g * GROUP_BLOCKS
                nb = gnb[g]
                W = nb * NODE_B
                jb = Jg[g]
                cap = cap_g[g]
                row0 = int(offg[g])
                cg0 = int(cog[g])

                lt_t = tbp.tile([P, nb * jb], bf16, tag="lt")
                nc.scalar.dma_start(out=lt_t[:], in_=lt_d[:, cg0:cg0 + nb * jb])

                # whole group's edges in ONE DMA; partition p holds, per block,
                # the contiguous run [row0 + bl*cap + p*jb, +jb)
                ga_t = gap.tile([P, nb * jb * F], bf16, tag="ga")
                nc.sync.dma_start(
                    out=ga_t[:].rearrange("p (b j f) -> p b j f", b=nb, j=jb),
                    in_=ea_d[row0:row0 + rows_g[g], :].rearrange(
                        "(b p j) f -> p b j f", p=P, j=jb))

                # replicate recip across partitions: PE ones.T @ recip_row
                rr_ps = y2ps.tile([P, W], f32, tag="rrps")
                nc.tensor.matmul(out=rr_ps[:], lhsT=ones_t[:],
                                 rhs=rcrow_t[:, gb0 * NODE_B:gb0 * NODE_B + W],
                                 start=True, stop=True)
                rr_t = mp.tile([P, W], f32, tag="rr")
                nc.scalar.copy(out=rr_t[:], in_=rr_ps[:])

                agg_ps = aps.tile([P, W], f32, tag="agg")
                for bl in range(nb):
                    cb0 = bl * jb
                    # one-hot for the whole block in ONE DVE op:
                    # oh[p, j, n] = (iota[n] == lt[p, cb0+j])
                    oh_t = ohp.tile([P, jb * NODE_B], bf16, tag="oh")
                    nc.vector.tensor_tensor(
                        out=oh_t[:],
                        in0=iota_t[:, :jb * NODE_B],
                        in1=lt_t[:, cb0:cb0 + jb, None].to_broadcast(
                            [P, jb, NODE_B]),
                        op=mybir.AluOpType.is_equal)
                    for i in range(jb):
                        nc.tensor.matmul(
                            out=agg_ps[:, bl * NODE_B:(bl + 1) * NODE_B],
                            lhsT=ga_t[:, (bl * jb + i) * P:(bl * jb + i + 1) * P],
                            rhs=oh_t[:, i * NODE_B:(i + 1) * NODE_B],
                            start=(i == 0), stop=(i == jb - 1))

                # scale by recip while evicting PSUM -> SBUF (one DVE op)
                aggT_sb = mp.tile([P, W], mdt, tag="aggT")
                nc.vector.tensor_tensor(
                    out=aggT_sb[:], in0=agg_ps[:], in1=rr_t[:],
                    op=mybir.AluOpType.mult)

                # ---- MLP over this group's W nodes (transposed layout) ----
                xt_sb = mp.tile([P, W], mdt, tag="xt")
                nc.scalar.dma_start(out=xt_sb[:],
                                    in_=xt_d[:, gb0 * NODE_B:gb0 * NODE_B + W])

                y1_sb = []
                for oh in range(2):
                    y1_ps = y1ps.tile([P, W], f32, tag=f"y1_{oh}")
                    nc.tensor.matmul(out=y1_ps[:], lhsT=w1t_t[0][:, oh * P:(oh + 1) * P],
                                     rhs=xt_sb[:], start=True, stop=False)
                    nc.tensor.matmul(out=y1_ps[:], lhsT=w1t_t[1][:, oh * P:(oh + 1) * P],
                                     rhs=aggT_sb[:], start=False, stop=True)
                    y1c = mp.tile([P, W], mdt, tag=f"y1sb{oh}", name=f"y1c{oh}")
                    nc.scalar.activation(out=y1c[:], in_=y1_ps[:],
                                         func=mybir.ActivationFunctionType.Relu,
                                         bias=b1_t[oh][:])
                    y1_sb.append(y1c)

                y2_ps = y2ps.tile([P, W], f32, tag="y2")
                nc.tensor.matmul(out=y2_ps[:], lhsT=w2t_t[0][:], rhs=y1_sb[0][:],
                                 start=True, stop=False)
                nc.tensor.matmul(out=y2_ps[:], lhsT=w2t_t[1][:], rhs=y1_sb[1][:],
                                 start=False, stop=True)
                y2_sb = mp.tile([P, W], f32, tag="y2sb")
                nc.scalar.activation(out=y2_sb[:], in_=y2_ps[:],
                                     func=mybir.ActivationFunctionType.Identity,
                                     bias=b2_t[:])
                nc.scalar.dma_start(out=out_d[:, gb0 * NODE_B:gb0 * NODE_B + W],
                                    in_=y2_sb[:])

    nc.compile()
    return nc


def kernel(x, edge_index, edge_attr, W1, b1, W2, b2, _trace=False):
    global LAST_EXEC_NS, LAST_RESULTS
    from concourse.bass_utils import run_bass_kernel_spmd

    in_maps, params = _preprocess(x, edge_index, edge_attr, W1, b1, W2, b2)
    if params not in _COMPILED:
        _COMPILED[params] = _build(params)
    nc = _COMPILED[params]

    res = run_bass_kernel_spmd(nc, in_maps, core_ids=list(range(N_CORES)),
                               trace=_trace)
    LAST_EXEC_NS = res.exec_time_ns
    LAST_RESULTS = res
    out = np.empty((N_NODES, OUT_F), np.float32)
    for c, r in enumerate(res.results):
        out[c * NODES_PER_CORE:(c + 1) * NODES_PER_CORE] = \
            r["out"][:, :NODES_PER_CORE].T
    return out



# revision 3
# speedup vs baseline: 1.2312x; 1.2312x over previous
"""Trainium2 Bass kernel for GNN aggregate-update (scatter-mean + concat + MLP).

Strategy (8 NeuronCores, SPMD, no collectives):
  - Host routing: sort edges by target node, bucket nodes by degree into
    capacity classes (C in {4,8,12,...,128}); each node's edge run is padded
    to its capacity. Nodes are dealt round-robin per class across the 8
    cores, so every core has the SAME static chunk schedule (one NEFF).
  - A "chunk" is 128 edge slots on the 128 SBUF partitions holding
    npc = floor(128/C) nodes of one class, each node occupying C
    consecutive partition rows. The scatter-sum for a chunk is ONE PE
    matmul: lhsT = attr chunk [128e, 128f] (fp8, stationary, full-column
    -> fast weight load), rhs = a per-class CONSTANT block-diagonal 0/1
    pattern [128e, npc] (fp8). No per-edge one-hot is ever built on DVE.
  - Edge features are fp8 e4m3 (host-quantized): halves the dominant HBM
    stream vs bf16 and halves PE weight-load time.
  - recip = 1/max(degree,1) is replicated across partitions by a K=1 PE
    matmul (ones.T @ recip_row) and applied by one DVE multiply per group
    while evicting the aggregate PSUM->SBUF.
  - MLP in transposed layout (features on partitions): y1T = relu(W1T.T @
    [xT; aggT] + b1), y2T = W2T.T @ y1T + b2, biases applied by ACT at
    PSUM eviction. MLP operands bf16, PSUM f32, output f32.
  - Output stays transposed [128, nodes-in-bucketed-order]; the host
    scatters columns back to original node ids while unsharding.
"""

import numpy as np
import ml_dtypes

N_NODES = 100_000
N_EDGES = 1_600_000
F = 128
HIDDEN = 256
OUT_F = 128
N_CORES = 8
P = 128
GROUP_W = 512          # max nodes per MLP group (one PSUM bank)
MAX_CH = 128           # max chunks per group (SBUF tile cap)

# (capacity, nodes-per-chunk); capacity*npc <= 128
CAPS = [(4, 32), (8, 16), (12, 10), (16, 8), (20, 6), (24, 5),
        (32, 4), (42, 3), (64, 2), (128, 1)]
PAT_OFF = np.concatenate([[0], np.cumsum([npc for _, npc in CAPS])]).astype(int)
PAT_W = int(PAT_OFF[-1])

BF16 = ml_dtypes.bfloat16
FP8 = ml_dtypes.float8_e4m3

_COMPILED = {}
LAST_EXEC_NS = None
LAST_RESULTS = None


def _make_schedule(chunks_per_class):
    """Greedy chunk->group packing, shared by host and device builder."""
    chunk_ci = np.repeat(np.arange(len(CAPS)), chunks_per_class)
    groups = []  # (k0, nch, W, node_off)
    k0, W, noff = 0, 0, 0
    for k, ci in enumerate(chunk_ci):
        n = CAPS[ci][1]
        if W + n > GROUP_W or (k - k0) >= MAX_CH:
            groups.append((k0, k - k0, W, noff))
            noff += W
            k0, W = k, 0
        W += n
    if W:
        groups.append((k0, len(chunk_ci) - k0, W, noff))
        noff += W
    return chunk_ci, groups, noff  # noff == NLOC


def _preprocess(x, edge_index, edge_attr, W1, b1, W2, b2):
    col = np.asarray(edge_index[1]).astype(np.int64)
    order = np.argsort(col, kind="stable")
    sorted_col = col[order]
    counts = np.bincount(col, minlength=N_NODES).astype(np.int64)
    start = np.searchsorted(sorted_col, np.arange(N_NODES), side="left")
    recip = (1.0 / np.maximum(counts, 1)).astype(np.float32)

    dmax = np.maximum(counts, 1)
    assert dmax.max() <= CAPS[-1][0], f"degree {dmax.max()} exceeds max capacity"
    cls = np.full(N_NODES, len(CAPS) - 1, np.int64)
    for ci in range(len(CAPS) - 1, -1, -1):
        cls[dmax <= CAPS[ci][0]] = ci

    # deal nodes per class round-robin across cores; pad to full chunks
    chunks_per_class = []
    core_nodes = [[] for _ in range(N_CORES)]
    for ci, (C, npc) in enumerate(CAPS):
        ids = np.where(cls == ci)[0]
        m = -(-len(ids) // N_CORES) if len(ids) else 0
        ch = -(-m // npc) if m else 0
        chunks_per_class.append(ch)
        M = ch * npc
        for c in range(N_CORES):
            sel = ids[c::N_CORES]
            a = np.full(M, -1, np.int64)
            a[: len(sel)] = sel
            core_nodes[c].append(a)
    chunks_per_class = tuple(chunks_per_class)
    core_nodes = [np.concatenate(l) if l else np.empty(0, np.int64)
                  for l in core_nodes]

    chunk_ci, groups, NLOC = _make_schedule(chunks_per_class)
    TOTCH = len(chunk_ci)

    # per node position: chunk index and base partition row
    pos_k = np.empty(NLOC, np.int64)
    pos_row = np.empty(NLOC, np.int64)
    off_n, off_k = 0, 0
    for ci, (C, npc) in enumerate(CAPS):
        ch = chunks_per_class[ci]
        if not ch:
            continue
        M = ch * npc
        t = np.arange(M)
        pos_k[off_n:off_n + M] = off_k + t // npc
        pos_row[off_n:off_n + M] = (t % npc) * C
        off_n += M
        off_k += ch

    ea8 = np.asarray(edge_attr, np.float32).astype(FP8)
    xt_full = np.ascontiguousarray(np.asarray(x, np.float32).T.astype(BF16))

    # per-class constant block-diagonal patterns, packed into one table
    pat = np.zeros((P, PAT_W), FP8)
    for ci, (C, npc) in enumerate(CAPS):
        o = PAT_OFF[ci]
        for j in range(npc):
            pat[j * C:(j + 1) * C, o + j] = 1.0

    w1t = np.ascontiguousarray(np.asarray(W1, np.float32).T).astype(BF16)
    w2t = np.ascontiguousarray(np.asarray(W2, np.float32).T).astype(BF16)

    in_maps, col2nid = [], []
    for c in range(N_CORES):
        gid = core_nodes[c]
        valid = gid >= 0
        gidc = np.where(valid, gid, 0)
        d = np.where(valid, counts[gidc], 0)
        s = np.where(valid, start[gidc], 0)
        slot_base = pos_k * P + pos_row
        E_c = int(d.sum())
        rep = np.repeat(np.arange(NLOC), d)
        within = np.arange(E_c) - np.repeat(np.cumsum(d) - d, d)
        rows = slot_base[rep] + within
        eids = order[np.repeat(s, d) + within]
        buf = np.zeros((TOTCH * P, F), FP8)
        buf[rows] = ea8[eids]
        attr = np.ascontiguousarray(
            buf.reshape(TOTCH, P, F).transpose(1, 0, 2).reshape(P, TOTCH * F))

        xt = np.zeros((F, NLOC), BF16)
        xt[:, valid] = xt_full[:, gid[valid]]
        rc = np.ones(NLOC, BF16)
        rc[valid] = recip[gid[valid]].astype(BF16)

        in_maps.append({
            "ea": attr,
            "pat": pat,
            "rcrow": np.ascontiguousarray(rc),
            "xT": np.ascontiguousarray(xt),
            "w1t": w1t,
            "w2t": w2t,
            "b1": np.asarray(b1, np.float32),
            "b2": np.asarray(b2, np.float32),
        })
        col2nid.append(gid)
    return in_maps, chunks_per_class, col2nid


def _build(params):
    """Build + compile the per-core Bass program (same NEFF for all cores)."""
    import concourse.bass as bass
    import concourse.bacc as bacc
    import concourse.tile as tile
    import concourse.mybir as mybir

    chunks_per_class = params
    chunk_ci, groups, NLOC = _make_schedule(chunks_per_class)
    TOTCH = len(chunk_ci)

    f32 = mybir.dt.float32
    bf16 = mybir.dt.bfloat16
    fp8 = mybir.dt.float8e4

    nc = bacc.Bacc("TRN2", target_bir_lowering=False, debug=False,
                   num_devices=N_CORES)
    ea_d = nc.dram_tensor("ea", [P, TOTCH * F], fp8, kind="ExternalInput").ap()
    pat_d = nc.dram_tensor("pat", [P, PAT_W], fp8, kind="ExternalInput").ap()
    rc_d = nc.dram_tensor("rcrow", [NLOC], bf16, kind="ExternalInput").ap()
    xt_d = nc.dram_tensor("xT", [F, NLOC], bf16, kind="ExternalInput").ap()
    w1t_d = nc.dram_tensor("w1t", [HIDDEN, HIDDEN], bf16, kind="ExternalInput").ap()
    w2t_d = nc.dram_tensor("w2t", [HIDDEN, OUT_F], bf16, kind="ExternalInput").ap()
    b1_d = nc.dram_tensor("b1", [HIDDEN], f32, kind="ExternalInput").ap()
    b2_d = nc.dram_tensor("b2", [OUT_F], f32, kind="ExternalInput").ap()
    out_d = nc.dram_tensor("out", [OUT_F, NLOC], f32, kind="ExternalOutput").ap()

    with tile.TileContext(nc) as tc:
        with (
            tc.tile_pool(name="const", bufs=1) as cp,
            tc.tile_pool(name="ga", bufs=3) as gap,
            tc.tile_pool(name="mlp", bufs=3) as mp,
            tc.tile_pool(name="agg_ps", bufs=2, space="PSUM") as aps,
            tc.tile_pool(name="y1_ps", bufs=2, space="PSUM") as y1ps,
            tc.tile_pool(name="y2_ps", bufs=1, space="PSUM") as y2ps,
        ):
            # ---- constants ----
            pat_t = cp.tile([P, PAT_W], fp8)
            nc.scalar.dma_start(out=pat_t[:], in_=pat_d[:])
            w1t_t = []
            for fc in range(2):
                w1c = cp.tile([P, HIDDEN], bf16, name=f"w1c{fc}")
                nc.scalar.dma_start(out=w1c[:], in_=w1t_d[fc * P:(fc + 1) * P, :])
                w1t_t.append(w1c)
            w2t_t = []
            for oc in range(2):
                w2c = cp.tile([P, OUT_F], bf16, name=f"w2c{oc}")
                nc.scalar.dma_start(out=w2c[:], in_=w2t_d[oc * P:(oc + 1) * P, :])
                w2t_t.append(w2c)
            b1_t = []
            for oh in range(2):
                b1c = cp.tile([P, 1], f32, name=f"b1c{oh}")
                nc.scalar.dma_start(out=b1c[:], in_=b1_d[oh * P:(oh + 1) * P, None])
                b1_t.append(b1c)
            b2_t = cp.tile([P, 1], f32)
            nc.scalar.dma_start(out=b2_t[:], in_=b2_d[:, None])
            ones_t = cp.tile([1, P], bf16)
            nc.vector.memset(ones_t[:], 1.0)
            rcrow_t = cp.tile([1, NLOC], bf16)
            nc.scalar.dma_start(out=rcrow_t[:], in_=rc_d[None, :])

            for (k0, nch, W, noff) in groups:
                # whole group's edge chunks in ONE contiguous DMA
                ga_t = gap.tile([P, nch * F], fp8, tag="ga")
                nc.sync.dma_start(out=ga_t[:], in_=ea_d[:, k0 * F:(k0 + nch) * F])

                # replicate recip across partitions: PE ones.T @ recip_row
                rr_ps = y2ps.tile([P, W], f32, tag="rrps")
                nc.tensor.matmul(out=rr_ps[:], lhsT=ones_t[:],
                                 rhs=rcrow_t[:, noff:noff + W],
                                 start=True, stop=True)
                rr_t = mp.tile([P, W], f32, tag="rr")
                nc.scalar.copy(out=rr_t[:], in_=rr_ps[:])

                # scatter-sum: one matmul per chunk against its class pattern
                agg_ps = aps.tile([P, W], f32, tag="agg")
                o = 0
                for lc in range(nch):
                    ci = int(chunk_ci[k0 + lc])
                    npc = CAPS[ci][1]
                    po = int(PAT_OFF[ci])
                    nc.tensor.matmul(
                        out=agg_ps[:, o:o + npc],
                        lhsT=ga_t[:, lc * F:(lc + 1) * F],
                        rhs=pat_t[:, po:po + npc],
                        start=True, stop=True)
                    o += npc
                assert o == W

                # scale by recip while evicting PSUM -> SBUF (one DVE op)
                aggT_sb = mp.tile([P, W], bf16, tag="aggT")
                nc.vector.tensor_tensor(
                    out=aggT_sb[:], in0=agg_ps[:], in1=rr_t[:],
                    op=mybir.AluOpType.mult)

                # ---- MLP over this group's W nodes (transposed layout) ----
                xt_sb = mp.tile([P, W], bf16, tag="xt")
                nc.gpsimd.dma_start(out=xt_sb[:], in_=xt_d[:, noff:noff + W])

                y1_sb = []
                for oh in range(2):
                    y1_ps = y1ps.tile([P, W], f32, tag=f"y1_{oh}")
                    nc.tensor.matmul(out=y1_ps[:], lhsT=w1t_t[0][:, oh * P:(oh + 1) * P],
                                     rhs=xt_sb[:], start=True, stop=False)
                    nc.tensor.matmul(out=y1_ps[:], lhsT=w1t_t[1][:, oh * P:(oh + 1) * P],
                                     rhs=aggT_sb[:], start=False, stop=True)
                    y1c = mp.tile([P, W], bf16, tag=f"y1sb{oh}", name=f"y1c{oh}")
                    nc.scalar.activation(out=y1c[:], in_=y1_ps[:],
                                         func=mybir.ActivationFunctionType.Relu,
                                         bias=b1_t[oh][:])
                    y1_sb.append(y1c)

                y2_ps = y2ps.tile([P, W], f32, tag="y2")
                nc.tensor.matmul(out=y2_ps[:], lhsT=w2t_t[0][:], rhs=y1_sb[0][:],
                                 start=True, stop=False)
                nc.tensor.matmul(out=y2_ps[:], lhsT=w2t_t[1][:], rhs=y1_sb[1][:],
                                 start=False, stop=True)
                y2_sb = mp.tile([P, W], f32, tag="y2sb")
                nc.scalar.activation(out=y2_sb[:], in_=y2_ps[:],
                                     func=mybir.ActivationFunctionType.Identity,
                                     bias=b2_t[:])
                nc.sync.dma_start(out=out_d[:, noff:noff + W], in_=y2_sb[:])

    nc.compile()
    return nc


def kernel(x, edge_index, edge_attr, W1, b1, W2, b2, _trace=False):
    global LAST_EXEC_NS, LAST_RESULTS
    from concourse.bass_utils import run_bass_kernel_spmd

    in_maps, params, col2nid = _preprocess(x, edge_index, edge_attr,
                                           W1, b1, W2, b2)
    if params not in _COMPILED:
        _COMPILED[params] = _build(params)
    nc = _COMPILED[params]

    res = run_bass_kernel_spmd(nc, in_maps, core_ids=list(range(N_CORES)),
                               trace=_trace)
    LAST_EXEC_NS = res.exec_time_ns
    LAST_RESULTS = res
    out = np.empty((N_NODES, OUT_F), np.float32)
    for c, r in enumerate(res.results):
        gid = col2nid[c]
        valid = gid >= 0
        out[gid[valid]] = r["out"][:, valid].T
    return out


# revision 6
# speedup vs baseline: 1.2583x; 1.0220x over previous
"""Trainium2 Bass kernel for GNN aggregate-update (scatter-mean + concat + MLP).

Strategy (8 NeuronCores, SPMD, no collectives):
  - Host routing: sort edges by target node, bucket nodes by degree into
    capacity classes (C in {4,8,12,...,128}); each node's edge run is padded
    to its capacity. Nodes are dealt round-robin per class across the 8
    cores, so every core has the SAME static chunk schedule (one NEFF).
  - A "chunk" is 128 edge slots on the 128 SBUF partitions holding
    npc = floor(128/C) nodes of one class, each node occupying C
    consecutive partition rows. The scatter-sum for a chunk is ONE PE
    matmul: lhsT = attr chunk [128e, 128f] (fp8, stationary, full-column
    -> fast weight load), rhs = a per-class CONSTANT block-diagonal 0/1
    pattern [128e, npc] (fp8). No per-edge one-hot is ever built on DVE.
  - Edge features are fp8 e4m3 (host-quantized): halves the dominant HBM
    stream vs bf16 and halves PE weight-load time.
  - recip = 1/max(degree,1) is replicated across partitions by a K=1 PE
    matmul (ones.T @ recip_row) and applied by one DVE multiply per group
    while evicting the aggregate PSUM->SBUF.
  - MLP in transposed layout (features on partitions): y1T = relu(W1T.T @
    [xT; aggT] + b1), y2T = W2T.T @ y1T + b2, biases applied by ACT at
    PSUM eviction. MLP operands bf16, PSUM f32, output f32.
  - Output stays transposed [128, nodes-in-bucketed-order]; the host
    scatters columns back to original node ids while unsharding.
"""

import numpy as np
import ml_dtypes

N_NODES = 100_000
N_EDGES = 1_600_000
F = 128
HIDDEN = 256
OUT_F = 128
N_CORES = 8
P = 128
GROUP_W = 512          # max nodes per MLP group (one PSUM bank)
MAX_CH = 128           # max chunks per group (SBUF tile cap)

# (capacity, nodes-per-chunk); capacity*npc <= 128
CAPS = [(4, 32), (8, 16), (12, 10), (16, 8), (20, 6), (24, 5),
        (32, 4), (42, 3), (64, 2), (128, 1)]
PAT_OFF = np.concatenate([[0], np.cumsum([npc for _, npc in CAPS])]).astype(int)
PAT_W = int(PAT_OFF[-1])

BF16 = ml_dtypes.bfloat16
FP8 = ml_dtypes.float8_e3m4

_COMPILED = {}
LAST_EXEC_NS = None
LAST_RESULTS = None


def _make_schedule(chunks_per_class):
    """Greedy chunk->group packing, shared by host and device builder."""
    chunk_ci = np.repeat(np.arange(len(CAPS)), chunks_per_class)
    groups = []  # (k0, nch, W, node_off)
    k0, W, noff = 0, 0, 0
    for k, ci in enumerate(chunk_ci):
        n = CAPS[ci][1]
        if W + n > GROUP_W or (k - k0) >= MAX_CH:
            groups.append((k0, k - k0, W, noff))
            noff += W
            k0, W = k, 0
        W += n
    if W:
        groups.append((k0, len(chunk_ci) - k0, W, noff))
        noff += W
    return chunk_ci, groups, noff  # noff == NLOC


def _preprocess(x, edge_index, edge_attr, W1, b1, W2, b2):
    col = np.asarray(edge_index[1]).astype(np.int64)
    order = np.argsort(col, kind="stable")
    sorted_col = col[order]
    counts = np.bincount(col, minlength=N_NODES).astype(np.int64)
    start = np.searchsorted(sorted_col, np.arange(N_NODES), side="left")
    recip = (1.0 / np.maximum(counts, 1)).astype(np.float32)

    dmax = np.maximum(counts, 1)
    assert dmax.max() <= CAPS[-1][0], f"degree {dmax.max()} exceeds max capacity"
    cls = np.full(N_NODES, len(CAPS) - 1, np.int64)
    for ci in range(len(CAPS) - 1, -1, -1):
        cls[dmax <= CAPS[ci][0]] = ci

    # deal nodes per class round-robin across cores; pad to full chunks
    chunks_per_class = []
    core_nodes = [[] for _ in range(N_CORES)]
    for ci, (C, npc) in enumerate(CAPS):
        ids = np.where(cls == ci)[0]
        m = -(-len(ids) // N_CORES) if len(ids) else 0
        ch = -(-m // npc) if m else 0
        chunks_per_class.append(ch)
        M = ch * npc
        for c in range(N_CORES):
            sel = ids[c::N_CORES]
            a = np.full(M, -1, np.int64)
            a[: len(sel)] = sel
            core_nodes[c].append(a)
    chunks_per_class = tuple(chunks_per_class)
    core_nodes = [np.concatenate(l) if l else np.empty(0, np.int64)
                  for l in core_nodes]

    chunk_ci, groups, NLOC = _make_schedule(chunks_per_class)
    TOTCH = len(chunk_ci)

    # per node position: chunk index and base partition row
    pos_k = np.empty(NLOC, np.int64)
    pos_row = np.empty(NLOC, np.int64)
    off_n, off_k = 0, 0
    for ci, (C, npc) in enumerate(CAPS):
        ch = chunks_per_class[ci]
        if not ch:
            continue
        M = ch * npc
        t = np.arange(M)
        pos_k[off_n:off_n + M] = off_k + t // npc
        pos_row[off_n:off_n + M] = (t % npc) * C
        off_n += M
        off_k += ch

    ea8 = np.asarray(edge_attr, np.float32).astype(FP8)
    xt_full = np.ascontiguousarray(np.asarray(x, np.float32).T.astype(BF16))

    # per-class constant block-diagonal patterns, packed into one table
    pat = np.zeros((P, PAT_W), FP8)
    for ci, (C, npc) in enumerate(CAPS):
        o = PAT_OFF[ci]
        for j in range(npc):
            pat[j * C:(j + 1) * C, o + j] = 1.0

    w1t = np.ascontiguousarray(np.asarray(W1, np.float32).T).astype(BF16)
    w2t = np.ascontiguousarray(np.asarray(W2, np.float32).T).astype(BF16)

    in_maps, col2nid = [], []
    for c in range(N_CORES):
        gid = core_nodes[c]
        valid = gid >= 0
        gidc = np.where(valid, gid, 0)
        d = np.where(valid, counts[gidc], 0)
        s = np.where(valid, start[gidc], 0)
        slot_base = pos_k * P + pos_row
        E_c = int(d.sum())
        rep = np.repeat(np.arange(NLOC), d)
        within = np.arange(E_c) - np.repeat(np.cumsum(d) - d, d)
        rows = slot_base[rep] + within
        eids = order[np.repeat(s, d) + within]
        buf = np.zeros((TOTCH * P, F), FP8)
        buf[rows] = ea8[eids]
        attr = np.ascontiguousarray(
            buf.reshape(TOTCH, P, F).transpose(1, 0, 2).reshape(P, TOTCH * F))

        xt = np.zeros((F, NLOC), BF16)
        xt[:, valid] = xt_full[:, gid[valid]]
        rc = np.ones(NLOC, BF16)
        rc[valid] = recip[gid[valid]].astype(BF16)

        in_maps.append({
            "ea": attr,
            "pat": pat,
            "rcrow": np.ascontiguousarray(rc),
            "xT": np.ascontiguousarray(xt),
            "w1t": w1t,
            "w2t": w2t,
            "b1": np.asarray(b1, np.float32),
            "b2": np.asarray(b2, np.float32),
        })
        col2nid.append(gid)
    return in_maps, chunks_per_class, col2nid


def _build(params):
    """Build + compile the per-core Bass program (same NEFF for all cores)."""
    import concourse.bass as bass
    import concourse.bacc as bacc
    import concourse.tile as tile
    import concourse.mybir as mybir

    chunks_per_class = params
    chunk_ci, groups, NLOC = _make_schedule(chunks_per_class)
    TOTCH = len(chunk_ci)

    f32 = mybir.dt.float32
    bf16 = mybir.dt.bfloat16
    fp8 = mybir.dt.float8e3

    nc = bacc.Bacc("TRN2", target_bir_lowering=False, debug=False,
                   num_devices=N_CORES)
    ea_d = nc.dram_tensor("ea", [P, TOTCH * F], fp8, kind="ExternalInput").ap()
    pat_d = nc.dram_tensor("pat", [P, PAT_W], fp8, kind="ExternalInput").ap()
    rc_d = nc.dram_tensor("rcrow", [NLOC], bf16, kind="ExternalInput").ap()
    xt_d = nc.dram_tensor("xT", [F, NLOC], bf16, kind="ExternalInput").ap()
    w1t_d = nc.dram_tensor("w1t", [HIDDEN, HIDDEN], bf16, kind="ExternalInput").ap()
    w2t_d = nc.dram_tensor("w2t", [HIDDEN, OUT_F], bf16, kind="ExternalInput").ap()
    b1_d = nc.dram_tensor("b1", [HIDDEN], f32, kind="ExternalInput").ap()
    b2_d = nc.dram_tensor("b2", [OUT_F], f32, kind="ExternalInput").ap()
    out_d = nc.dram_tensor("out", [OUT_F, NLOC], f32, kind="ExternalOutput").ap()

    with tile.TileContext(nc) as tc:
        with (
            tc.tile_pool(name="const", bufs=1) as cp,
            tc.tile_pool(name="ga", bufs=3) as gap,
            tc.tile_pool(name="mlp", bufs=3) as mp,
            tc.tile_pool(name="agg_ps", bufs=2, space="PSUM") as aps,
            tc.tile_pool(name="y1_ps", bufs=2, space="PSUM") as y1ps,
            tc.tile_pool(name="y2_ps", bufs=1, space="PSUM") as y2ps,
        ):
            # ---- constants ----
            pat_t = cp.tile([P, PAT_W], fp8)
            nc.scalar.dma_start(out=pat_t[:], in_=pat_d[:])
            w1t_t = []
            for fc in range(2):
                w1c = cp.tile([P, HIDDEN], bf16, name=f"w1c{fc}")
                nc.scalar.dma_start(out=w1c[:], in_=w1t_d[fc * P:(fc + 1) * P, :])
                w1t_t.append(w1c)
            w2t_t = []
            for oc in range(2):
                w2c = cp.tile([P, OUT_F], bf16, name=f"w2c{oc}")
                nc.scalar.dma_start(out=w2c[:], in_=w2t_d[oc * P:(oc + 1) * P, :])
                w2t_t.append(w2c)
            b1_t = []
            for oh in range(2):
                b1c = cp.tile([P, 1], f32, name=f"b1c{oh}")
                nc.scalar.dma_start(out=b1c[:], in_=b1_d[oh * P:(oh + 1) * P, None])
                b1_t.append(b1c)
            b2_t = cp.tile([P, 1], f32)
            nc.scalar.dma_start(out=b2_t[:], in_=b2_d[:, None])
            ones_t = cp.tile([1, P], bf16)
            nc.vector.memset(ones_t[:], 1.0)
            rcrow_t = cp.tile([1, NLOC], bf16)
            nc.scalar.dma_start(out=rcrow_t[:], in_=rc_d[None, :])

            def emit_mlp(W, noff, xt_sb, aggT_sb):
                """MLP over one group's W nodes (transposed layout)."""
                y1_sb = []
                for oh in range(2):
                    y1_ps = y1ps.tile([P, W], f32, tag=f"y1_{oh}")
                    nc.tensor.matmul(out=y1_ps[:], lhsT=w1t_t[0][:, oh * P:(oh + 1) * P],
                                     rhs=xt_sb[:], start=True, stop=False)
                    nc.tensor.matmul(out=y1_ps[:], lhsT=w1t_t[1][:, oh * P:(oh + 1) * P],
                                     rhs=aggT_sb[:], start=False, stop=True)
                    y1c = mp.tile([P, W], bf16, tag=f"y1sb{oh}", name=f"y1c{oh}")
                    nc.scalar.activation(out=y1c[:], in_=y1_ps[:],
                                         func=mybir.ActivationFunctionType.Relu,
                                         bias=b1_t[oh][:])
                    y1_sb.append(y1c)

                y2_ps = y2ps.tile([P, W], f32, tag="y2")
                nc.tensor.matmul(out=y2_ps[:], lhsT=w2t_t[0][:], rhs=y1_sb[0][:],
                                 start=True, stop=False)
                nc.tensor.matmul(out=y2_ps[:], lhsT=w2t_t[1][:], rhs=y1_sb[1][:],
                                 start=False, stop=True)
                y2_sb = mp.tile([P, W], f32, tag="y2sb")
                nc.scalar.activation(out=y2_sb[:], in_=y2_ps[:],
                                     func=mybir.ActivationFunctionType.Identity,
                                     bias=b2_t[:])
                nc.gpsimd.dma_start(out=out_d[:, noff:noff + W], in_=y2_sb[:])

            # Software-pipelined: group g's MLP is emitted after group g+1's
            # aggregation matmuls so the in-order PE never waits on the
            # PSUM->SBUF eviction round-trip.
            pend = None
            for (k0, nch, W, noff) in groups:
                # whole group's edge chunks in ONE contiguous DMA
                ga_t = gap.tile([P, nch * F], fp8, tag="ga")
                nc.sync.dma_start(out=ga_t[:], in_=ea_d[:, k0 * F:(k0 + nch) * F])

                # replicate recip across partitions: PE ones.T @ recip_row
                rr_ps = y2ps.tile([P, W], f32, tag="rrps")
                nc.tensor.matmul(out=rr_ps[:], lhsT=ones_t[:],
                                 rhs=rcrow_t[:, noff:noff + W],
                                 start=True, stop=True)
                rr_t = mp.tile([P, W], f32, tag="rr")
                nc.vector.tensor_scalar_mul(rr_t[:], rr_ps[:], 1.0)

                # scatter-sum: one matmul per chunk against its class pattern
                agg_ps = aps.tile([P, W], f32, tag="agg")
                o = 0
                for lc in range(nch):
                    ci = int(chunk_ci[k0 + lc])
                    npc = CAPS[ci][1]
                    po = int(PAT_OFF[ci])
                    nc.tensor.matmul(
                        out=agg_ps[:, o:o + npc],
                        lhsT=ga_t[:, lc * F:(lc + 1) * F],
                        rhs=pat_t[:, po:po + npc],
                        start=True, stop=True)
                    o += npc
                assert o == W

                # scale by recip while evicting PSUM -> SBUF (one DVE op)
                aggT_sb = mp.tile([P, W], bf16, tag="aggT")
                nc.vector.tensor_tensor(
                    out=aggT_sb[:], in0=agg_ps[:], in1=rr_t[:],
                    op=mybir.AluOpType.mult)

                xt_sb = mp.tile([P, W], bf16, tag="xt")
                nc.gpsimd.dma_start(out=xt_sb[:], in_=xt_d[:, noff:noff + W])

                if pend is not None:
                    emit_mlp(*pend)
                pend = (W, noff, xt_sb, aggT_sb)
            emit_mlp(*pend)

    nc.compile()
    return nc


def kernel(x, edge_index, edge_attr, W1, b1, W2, b2, _trace=False):
    global LAST_EXEC_NS, LAST_RESULTS
    from concourse.bass_utils import run_bass_kernel_spmd

    in_maps, params, col2nid = _preprocess(x, edge_index, edge_attr,
                                           W1, b1, W2, b2)
    if params not in _COMPILED:
        _COMPILED[params] = _build(params)
    nc = _COMPILED[params]

    res = run_bass_kernel_spmd(nc, in_maps, core_ids=list(range(N_CORES)),
                               trace=_trace)
    LAST_EXEC_NS = res.exec_time_ns
    LAST_RESULTS = res
    out = np.empty((N_NODES, OUT_F), np.float32)
    for c, r in enumerate(res.results):
        gid = col2nid[c]
        valid = gid >= 0
        out[gid[valid]] = r["out"][:, valid].T
    return out


# revision 10
# speedup vs baseline: 1.3085x; 1.0399x over previous
"""Trainium2 Bass kernel for GNN aggregate-update (scatter-mean + concat + MLP).

Strategy (8 NeuronCores, SPMD, no collectives):
  - Host routing: sort edges by target node, bucket nodes by degree into
    capacity classes (C in {4,8,12,...,128}); each node's edge run is padded
    to its capacity. Nodes are dealt round-robin per class across the 8
    cores, so every core has the SAME static chunk schedule (one NEFF).
  - A "chunk" is 128 edge slots on the 128 SBUF partitions holding
    npc = floor(128/C) nodes of one class, each node occupying C
    consecutive partition rows. The scatter-sum for a chunk is ONE PE
    matmul: lhsT = attr chunk [128e, 128f] (fp8, stationary, full-column
    -> fast weight load), rhs = a per-class CONSTANT block-diagonal 0/1
    pattern [128e, npc] (fp8). No per-edge one-hot is ever built on DVE.
  - Edge features are fp8 e4m3 (host-quantized): halves the dominant HBM
    stream vs bf16 and halves PE weight-load time.
  - recip = 1/max(degree,1) is replicated across partitions by a K=1 PE
    matmul (ones.T @ recip_row) and applied by one DVE multiply per group
    while evicting the aggregate PSUM->SBUF.
  - MLP in transposed layout (features on partitions): y1T = relu(W1T.T @
    [xT; aggT] + b1), y2T = W2T.T @ y1T + b2, biases applied by ACT at
    PSUM eviction. MLP operands bf16, PSUM f32, output f32.
  - Output stays transposed [128, nodes-in-bucketed-order]; the host
    scatters columns back to original node ids while unsharding.
"""

import numpy as np
import ml_dtypes

N_NODES = 100_000
N_EDGES = 1_600_000
F = 128
HIDDEN = 256
OUT_F = 128
N_CORES = 8
P = 128
GROUP_W = 512          # max nodes per MLP group (one PSUM bank)
MAX_CH = 128           # max chunks per group (SBUF tile cap)

# (capacity, nodes-per-chunk); capacity*npc <= 128
CAPS = [(2, 64), (4, 32), (6, 21), (8, 16), (10, 12), (12, 10), (14, 9),
        (16, 8), (18, 7), (20, 6), (24, 5), (32, 4), (42, 3), (64, 2),
        (128, 1)]
PAT_OFF = np.concatenate([[0], np.cumsum([npc for _, npc in CAPS])]).astype(int)
PAT_W = int(PAT_OFF[-1])

BF16 = ml_dtypes.bfloat16
FP8 = ml_dtypes.float8_e3m4

_COMPILED = {}
LAST_EXEC_NS = None
LAST_RESULTS = None


def _make_schedule(chunks_per_class):
    """Greedy chunk->group packing, shared by host and device builder."""
    chunk_ci = np.repeat(np.arange(len(CAPS)), chunks_per_class)
    groups = []  # (k0, nch, W, node_off)
    k0, W, noff = 0, 0, 0
    for k, ci in enumerate(chunk_ci):
        n = CAPS[ci][1]
        if W + n > GROUP_W or (k - k0) >= MAX_CH:
            groups.append((k0, k - k0, W, noff))
            noff += W
            k0, W = k, 0
        W += n
    if W:
        groups.append((k0, len(chunk_ci) - k0, W, noff))
        noff += W
    return chunk_ci, groups, noff  # noff == NLOC


def _preprocess(x, edge_index, edge_attr, W1, b1, W2, b2):
    col = np.asarray(edge_index[1]).astype(np.int64)
    order = np.argsort(col, kind="stable")
    sorted_col = col[order]
    counts = np.bincount(col, minlength=N_NODES).astype(np.int64)
    start = np.searchsorted(sorted_col, np.arange(N_NODES), side="left")
    recip = (1.0 / np.maximum(counts, 1)).astype(np.float32)

    dmax = np.maximum(counts, 1)
    assert dmax.max() <= CAPS[-1][0], f"degree {dmax.max()} exceeds max capacity"
    cls = np.full(N_NODES, len(CAPS) - 1, np.int64)
    for ci in range(len(CAPS) - 1, -1, -1):
        cls[dmax <= CAPS[ci][0]] = ci

    # deal nodes per class round-robin across cores; pad to full chunks
    chunks_per_class = []
    core_nodes = [[] for _ in range(N_CORES)]
    for ci, (C, npc) in enumerate(CAPS):
        ids = np.where(cls == ci)[0]
        m = -(-len(ids) // N_CORES) if len(ids) else 0
        ch = -(-m // npc) if m else 0
        chunks_per_class.append(ch)
        M = ch * npc
        for c in range(N_CORES):
            sel = ids[c::N_CORES]
            a = np.full(M, -1, np.int64)
            a[: len(sel)] = sel
            core_nodes[c].append(a)
    chunks_per_class = tuple(chunks_per_class)
    core_nodes = [np.concatenate(l) if l else np.empty(0, np.int64)
                  for l in core_nodes]

    chunk_ci, groups, NLOC = _make_schedule(chunks_per_class)
    TOTCH = len(chunk_ci)

    # per node position: chunk index and base partition row
    pos_k = np.empty(NLOC, np.int64)
    pos_row = np.empty(NLOC, np.int64)
    off_n, off_k = 0, 0
    for ci, (C, npc) in enumerate(CAPS):
        ch = chunks_per_class[ci]
        if not ch:
            continue
        M = ch * npc
        t = np.arange(M)
        pos_k[off_n:off_n + M] = off_k + t // npc
        pos_row[off_n:off_n + M] = (t % npc) * C
        off_n += M
        off_k += ch

    ea8 = np.asarray(edge_attr, np.float32).astype(FP8)
    xt_full = np.ascontiguousarray(np.asarray(x, np.float32).T.astype(BF16))

    # per-class constant block-diagonal patterns, packed into one table
    pat = np.zeros((P, PAT_W), FP8)
    for ci, (C, npc) in enumerate(CAPS):
        o = PAT_OFF[ci]
        for j in range(npc):
            pat[j * C:(j + 1) * C, o + j] = 1.0

    w1t = np.ascontiguousarray(np.asarray(W1, np.float32).T).astype(BF16)
    w2t = np.ascontiguousarray(np.asarray(W2, np.float32).T).astype(BF16)

    in_maps, col2nid = [], []
    for c in range(N_CORES):
        gid = core_nodes[c]
        valid = gid >= 0
        gidc = np.where(valid, gid, 0)
        d = np.where(valid, counts[gidc], 0)
        s = np.where(valid, start[gidc], 0)
        slot_base = pos_k * P + pos_row
        E_c = int(d.sum())
        rep = np.repeat(np.arange(NLOC), d)
        within = np.arange(E_c) - np.repeat(np.cumsum(d) - d, d)
        rows = slot_base[rep] + within
        eids = order[np.repeat(s, d) + within]
        buf = np.zeros((TOTCH * P, F), FP8)
        buf[rows] = ea8[eids]
        attr = np.ascontiguousarray(
            buf.reshape(TOTCH, P, F).transpose(1, 0, 2).reshape(P, TOTCH * F))

        xt = np.zeros((F, NLOC), BF16)
        xt[:, valid] = xt_full[:, gid[valid]]
        rc = np.ones(NLOC, BF16)
        rc[valid] = recip[gid[valid]].astype(BF16)

        in_maps.append({
            "ea": attr,
            "pat": pat,
            "rcrow": np.ascontiguousarray(rc),
            "xT": np.ascontiguousarray(xt),
            "w1t": w1t,
            "w2t": w2t,
            "b1": np.asarray(b1, np.float32),
            "b2": np.asarray(b2, np.float32),
        })
        col2nid.append(gid)
    return in_maps, chunks_per_class, col2nid


def _build(params):
    """Build + compile the per-core Bass program (same NEFF for all cores)."""
    import concourse.bass as bass
    import concourse.bacc as bacc
    import concourse.tile as tile
    import concourse.mybir as mybir

    chunks_per_class = params
    chunk_ci, groups, NLOC = _make_schedule(chunks_per_class)
    TOTCH = len(chunk_ci)

    f32 = mybir.dt.float32
    bf16 = mybir.dt.bfloat16
    fp8 = mybir.dt.float8e3

    nc = bacc.Bacc("TRN2", target_bir_lowering=False, debug=False,
                   num_devices=N_CORES)
    ea_d = nc.dram_tensor("ea", [P, TOTCH * F], fp8, kind="ExternalInput").ap()
    pat_d = nc.dram_tensor("pat", [P, PAT_W], fp8, kind="ExternalInput").ap()
    rc_d = nc.dram_tensor("rcrow", [NLOC], bf16, kind="ExternalInput").ap()
    xt_d = nc.dram_tensor("xT", [F, NLOC], bf16, kind="ExternalInput").ap()
    w1t_d = nc.dram_tensor("w1t", [HIDDEN, HIDDEN], bf16, kind="ExternalInput").ap()
    w2t_d = nc.dram_tensor("w2t", [HIDDEN, OUT_F], bf16, kind="ExternalInput").ap()
    b1_d = nc.dram_tensor("b1", [HIDDEN], f32, kind="ExternalInput").ap()
    b2_d = nc.dram_tensor("b2", [OUT_F], f32, kind="ExternalInput").ap()
    out_d = nc.dram_tensor("out", [OUT_F, NLOC], bf16, kind="ExternalOutput").ap()

    with tile.TileContext(nc) as tc:
        with (
            tc.tile_pool(name="const", bufs=1) as cp,
            tc.tile_pool(name="ga", bufs=3) as gap,
            tc.tile_pool(name="mlp", bufs=3) as mp,
            tc.tile_pool(name="agg_ps", bufs=2, space="PSUM") as aps,
            tc.tile_pool(name="y1_ps", bufs=2, space="PSUM") as y1ps,
            tc.tile_pool(name="y2_ps", bufs=1, space="PSUM") as y2ps,
        ):
            # ---- constants ----
            pat_t = cp.tile([P, PAT_W], fp8)
            nc.scalar.dma_start(out=pat_t[:], in_=pat_d[:])
            w1t_t = []
            for fc in range(2):
                w1c = cp.tile([P, HIDDEN], bf16, name=f"w1c{fc}")
                nc.scalar.dma_start(out=w1c[:], in_=w1t_d[fc * P:(fc + 1) * P, :])
                w1t_t.append(w1c)
            w2t_t = []
            for oc in range(2):
                w2c = cp.tile([P, OUT_F], bf16, name=f"w2c{oc}")
                nc.scalar.dma_start(out=w2c[:], in_=w2t_d[oc * P:(oc + 1) * P, :])
                w2t_t.append(w2c)
            b1_t = []
            for oh in range(2):
                b1c = cp.tile([P, 1], f32, name=f"b1c{oh}")
                nc.scalar.dma_start(out=b1c[:], in_=b1_d[oh * P:(oh + 1) * P, None])
                b1_t.append(b1c)
            b2_t = cp.tile([P, 1], f32)
            nc.scalar.dma_start(out=b2_t[:], in_=b2_d[:, None])
            ones_t = cp.tile([1, P], bf16)
            nc.vector.memset(ones_t[:], 1.0)
            rcrow_t = cp.tile([1, NLOC], bf16)
            nc.scalar.dma_start(out=rcrow_t[:], in_=rc_d[None, :])

            def emit_mlp(W, noff, xt_sb, aggT_sb):
                """MLP over one group's W nodes (transposed layout)."""
                y1_sb = []
                for oh in range(2):
                    y1_ps = y1ps.tile([P, W], f32, tag=f"y1_{oh}")
                    nc.tensor.matmul(out=y1_ps[:], lhsT=w1t_t[0][:, oh * P:(oh + 1) * P],
                                     rhs=xt_sb[:], start=True, stop=False)
                    nc.tensor.matmul(out=y1_ps[:], lhsT=w1t_t[1][:, oh * P:(oh + 1) * P],
                                     rhs=aggT_sb[:], start=False, stop=True)
                    y1c = mp.tile([P, W], bf16, tag=f"y1sb{oh}", name=f"y1c{oh}")
                    nc.scalar.activation(out=y1c[:], in_=y1_ps[:],
                                         func=mybir.ActivationFunctionType.Relu,
                                         bias=b1_t[oh][:])
                    y1_sb.append(y1c)

                y2_ps = y2ps.tile([P, W], f32, tag="y2")
                nc.tensor.matmul(out=y2_ps[:], lhsT=w2t_t[0][:], rhs=y1_sb[0][:],
                                 start=True, stop=False)
                nc.tensor.matmul(out=y2_ps[:], lhsT=w2t_t[1][:], rhs=y1_sb[1][:],
                                 start=False, stop=True)
                y2_sb = mp.tile([P, W], bf16, tag="y2sb")
                nc.scalar.activation(out=y2_sb[:], in_=y2_ps[:],
                                     func=mybir.ActivationFunctionType.Identity,
                                     bias=b2_t[:])
                nc.gpsimd.dma_start(out=out_d[:, noff:noff + W], in_=y2_sb[:])

            # Software-pipelined: the PSUM->SBUF eviction of group g is
            # emitted one group late and the MLP two groups late, so the
            # in-order PE never waits on the eviction round-trip (the stalls
            # also keep the PE at its cold 1.2 GHz clock).
            ev_q, mlp_q = [], []
            for (k0, nch, W, noff) in groups:
                # whole group's edge chunks in ONE contiguous DMA
                ga_t = gap.tile([P, nch * F], fp8, tag="ga")
                nc.sync.dma_start(out=ga_t[:], in_=ea_d[:, k0 * F:(k0 + nch) * F])

                # replicate recip across partitions: PE ones.T @ recip_row
                rr_ps = y2ps.tile([P, W], f32, tag="rrps")
                nc.tensor.matmul(out=rr_ps[:], lhsT=ones_t[:],
                                 rhs=rcrow_t[:, noff:noff + W],
                                 start=True, stop=True)
                rr_t = mp.tile([P, W], f32, tag="rr")
                nc.vector.tensor_scalar_mul(rr_t[:], rr_ps[:], 1.0)

                if ev_q:
                    # evict the PREVIOUS group's aggregate (scale by recip)
                    pW, pnoff, p_agg_ps, p_rr_t, p_xt = ev_q.pop(0)
                    aggT_sb = mp.tile([P, pW], bf16, tag="aggT")
                    nc.vector.tensor_tensor(
                        out=aggT_sb[:], in0=p_agg_ps[:], in1=p_rr_t[:],
                        op=mybir.AluOpType.mult)
                    mlp_q.append((pW, pnoff, p_xt, aggT_sb))

                # scatter-sum: one matmul per chunk against its class pattern
                agg_ps = aps.tile([P, W], f32, tag="agg")
                o = 0
                for lc in range(nch):
                    ci = int(chunk_ci[k0 + lc])
                    npc = CAPS[ci][1]
                    po = int(PAT_OFF[ci])
                    nc.tensor.matmul(
                        out=agg_ps[:, o:o + npc],
                        lhsT=ga_t[:, lc * F:(lc + 1) * F],
                        rhs=pat_t[:, po:po + npc],
                        start=True, stop=True)
                    o += npc
                assert o == W

                xt_sb = mp.tile([P, W], bf16, tag="xt")
                nc.gpsimd.dma_start(out=xt_sb[:], in_=xt_d[:, noff:noff + W])
                ev_q.append((W, noff, agg_ps, rr_t, xt_sb))

                if len(mlp_q) >= 2:
                    emit_mlp(*mlp_q.pop(0))

            while ev_q:
                pW, pnoff, p_agg_ps, p_rr_t, p_xt = ev_q.pop(0)
                aggT_sb = mp.tile([P, pW], bf16, tag="aggT")
                nc.vector.tensor_tensor(
                    out=aggT_sb[:], in0=p_agg_ps[:], in1=p_rr_t[:],
                    op=mybir.AluOpType.mult)
                mlp_q.append((pW, pnoff, p_xt, aggT_sb))
            while mlp_q:
                emit_mlp(*mlp_q.pop(0))

    nc.compile()
    return nc


def kernel(x, edge_index, edge_attr, W1, b1, W2, b2, _trace=False):
    global LAST_EXEC_NS, LAST_RESULTS
    from concourse.bass_utils import run_bass_kernel_spmd

    in_maps, params, col2nid = _preprocess(x, edge_index, edge_attr,
                                           W1, b1, W2, b2)
    if params not in _COMPILED:
        _COMPILED[params] = _build(params)
    nc = _COMPILED[params]

    res = run_bass_kernel_spmd(nc, in_maps, core_ids=list(range(N_CORES)),
                               trace=_trace)
    LAST_EXEC_NS = res.exec_time_ns
    LAST_RESULTS = res
    out = np.empty((N_NODES, OUT_F), np.float32)
    for c, r in enumerate(res.results):
        gid = col2nid[c]
        valid = gid >= 0
        out[gid[valid]] = r["out"][:, valid].T.astype(np.float32)
    return out
